# revision 1
# baseline (speedup 1.0000x reference)
"""Trainium2 Bass kernel for nn_Bottleneck_5669356834470 (ResNet bottleneck
with an involution middle layer).

Sharding: data-parallel over batch. 16 samples / 8 cores = 2 samples/core.
All weights replicated (tiny).

Per-core pipeline (spatial 56x56 = 3136 flattened, S=2 samples):
  conv1 (1x1, 256->64) +BN1+ReLU   : PE matmuls (bf16), ACT evac w/ fused
                                     scale(folded)+bias+relu
  inv_c1 (1x1, 64->16) +BN+ReLU    : PE, ACT evac
  inv_c2 (1x1, 16->196) + bias     : PE, ACT evac -> dynamic weights w'
  involution (G=4, 7x7 dynamic)    : DVE tensor_tensor ops in a
      (sample, group, 4-row-chunk) partition layout (112 partitions); dynamic
      weights broadcast across the 16 group channels via a 0-stride AP dim;
      spatial shifts are free-dim slices of a zero-padded halo tensor.
  BN2+ReLU                         : ACT
  conv3 (1x1, 64->256) +BN3 + residual + ReLU : PE (residual folded in as an
      identity matmul over bf16 input), ACT evac w/ fused bias+relu.

Activation layout trick: activations are stored [channels, (sample, space)] so
every matmul operand sits at partition base 0 (no tile_position needed).

Compute dtype bf16 (f32 PSUM accumulation); output f32.
"""

import sys

sys.path.insert(0, "/opt/trn_rl_repo")

import numpy as np
import ml_dtypes

BF16 = ml_dtypes.bfloat16

S = 2            # samples per core
N_CORES = 8
CIN = 256
CMID = 64
G = 4            # involution groups
GC = 16          # channels per group
KS = 7           # involution kernel size
KK = KS * KS     # 49
R = 16           # dyn-weight bottleneck channels
H = W = 56
HW = H * W       # 3136
NCH = 7          # spatial chunks for matmul N dim
NW = HW // NCH   # 448
M = 14           # 4-row chunks per (sample, group)
RH = 4           # output rows per chunk
HR = 10          # halo rows stored per chunk (-3..+6)
WP = 62          # padded row width
NP_INV = S * G * M          # 112 involution partitions
XUF = GC * HR * W           # 8960 free elems per XU partition
XHF = GC * HR * WP          # 9920 free elems per XH partition
W2F = KK * RH * W           # 10976 free elems per W2 partition
ACCF = GC * RH * W          # 3584 acc free elems per partition
EPS = 1e-5

_CACHE = {}


def _ap(tile_ap, off, dims):
    """Raw strided AP on a tile's underlying tensor. dims=[(step,count),...]
    in elements; for SBUF the partition stride is ap[0][0] of the base AP."""
    import bass_rust

    return bass_rust.AP(tile_ap.tensor, tile_ap.offset + off, [list(d) for d in dims])


def build_module():
    if "nc" in _CACHE:
        return _CACHE["nc"]
    import concourse.bacc as bacc
    import concourse.mybir as mybir
    import concourse.tile as tile

    dt = mybir.dt
    AF = mybir.ActivationFunctionType

    nc = bacc.Bacc("TRN2", debug=False, num_devices=N_CORES)

    # ---- DRAM I/O ----------------------------------------------------------
    xin = nc.dram_tensor("xin", [S, CIN, HW], dt.float32, kind="ExternalInput")
    w1t = nc.dram_tensor("w1t", [2, 128, CMID], dt.bfloat16, kind="ExternalInput")
    b1 = nc.dram_tensor("b1", [CMID, 1], dt.float32, kind="ExternalInput")
    c1t = nc.dram_tensor("c1t", [CMID, R], dt.bfloat16, kind="ExternalInput")
    bi = nc.dram_tensor("bi", [R, 1], dt.float32, kind="ExternalInput")
    c2t = nc.dram_tensor("c2t", [R, G * KK], dt.bfloat16, kind="ExternalInput")
    b2ca = nc.dram_tensor("b2ca", [128, 1], dt.float32, kind="ExternalInput")
    b2cb = nc.dram_tensor("b2cb", [68, 1], dt.float32, kind="ExternalInput")
    s2v = nc.dram_tensor("s2v", [CMID, 1], dt.float32, kind="ExternalInput")
    b2v = nc.dram_tensor("b2v", [CMID, 1], dt.float32, kind="ExternalInput")
    w3t = nc.dram_tensor("w3t", [2, CMID, 128], dt.bfloat16, kind="ExternalInput")
    b3 = nc.dram_tensor("b3", [128, 2], dt.float32, kind="ExternalInput")
    ident = nc.dram_tensor("ident", [128, 128], dt.bfloat16, kind="ExternalInput")
    out = nc.dram_tensor("out", [S, CIN, HW], dt.float32, kind="ExternalOutput")

    with tile.TileContext(nc) as tc:
        with (
            tc.tile_pool(name="consts", bufs=1) as cpool,
            tc.tile_pool(name="big", bufs=1) as bpool,
            tc.tile_pool(name="psum", bufs=6, space="PSUM") as ppool,
            tc.tile_pool(name="stage", bufs=4) as spool,
            tc.tile_pool(name="dstage", bufs=1, space="DRAM") as dpool,
        ):
            # ---- constants -> SBUF ----------------------------------------
            w1t_sb = cpool.tile([128, 2 * CMID], dt.bfloat16, tag="w1t")
            nc.sync.dma_start(
                out=w1t_sb[:, :].rearrange("p (k c) -> p k c", k=2),
                in_=w1t.ap().rearrange("k p c -> p k c"),
            )
            b1_sb = cpool.tile([CMID, 1], dt.float32, tag="b1")
            nc.sync.dma_start(out=b1_sb[:, :], in_=b1.ap())
            c1t_sb = cpool.tile([CMID, R], dt.bfloat16, tag="c1t")
            nc.sync.dma_start(out=c1t_sb[:, :], in_=c1t.ap())
            bi_sb = cpool.tile([R, 1], dt.float32, tag="bi")
            nc.sync.dma_start(out=bi_sb[:, :], in_=bi.ap())
            c2t_sb = cpool.tile([R, G * KK], dt.bfloat16, tag="c2t")
            nc.sync.dma_start(out=c2t_sb[:, :], in_=c2t.ap())
            b2ca_sb = cpool.tile([128, 1], dt.float32, tag="b2ca")
            nc.sync.dma_start(out=b2ca_sb[:, :], in_=b2ca.ap())
            b2cb_sb = cpool.tile([68, 1], dt.float32, tag="b2cb")
            nc.sync.dma_start(out=b2cb_sb[:, :], in_=b2cb.ap())
            s2v_sb = cpool.tile([CMID, 1], dt.float32, tag="s2v")
            nc.sync.dma_start(out=s2v_sb[:, :], in_=s2v.ap())
            b2v_sb = cpool.tile([CMID, 1], dt.float32, tag="b2v")
            nc.sync.dma_start(out=b2v_sb[:, :], in_=b2v.ap())
            w3t_sb = cpool.tile([CMID, 2 * 128], dt.bfloat16, tag="w3t")
            nc.sync.dma_start(
                out=w3t_sb[:, :].rearrange("p (k c) -> p k c", k=2),
                in_=w3t.ap().rearrange("k p c -> p k c"),
            )
            b3_sb = cpool.tile([128, 2], dt.float32, tag="b3")
            nc.sync.dma_start(out=b3_sb[:, :], in_=b3.ap())
            id_sb = cpool.tile([128, 128], dt.bfloat16, tag="ident")
            nc.sync.dma_start(out=id_sb[:, :], in_=ident.ap())

            # ---- x load (f32 -> bf16 cast during SWDGE DMA) ---------------
            xbf = bpool.tile([128, S * 2 * HW], dt.bfloat16, tag="xbf")
            xbf_v = xbf[:, :].rearrange("p (s k f) -> p s k f", s=S, k=2)
            nc.gpsimd.dma_start(
                out=xbf_v,
                in_=xin.ap().rearrange("s (k p) f -> p s k f", p=128),
            )

            # ---- conv1 + BN1 + ReLU  -> out1 [64, (s, hw)] bf16 -----------
            out1 = bpool.tile([CMID, S * HW], dt.bfloat16, tag="out1")
            w1t_v = w1t_sb[:, :].rearrange("p (k c) -> p k c", k=2)
            for s in range(S):
                for n in range(NCH):
                    ps = ppool.tile([128, NW], dt.float32, tag="ps")
                    for kc in range(2):
                        nc.tensor.matmul(
                            ps[:CMID, :],
                            w1t_v[:, kc, :],
                            xbf_v[:, s, kc, n * NW : (n + 1) * NW],
                            start=(kc == 0),
                            stop=(kc == 1),
                        )
                    nc.scalar.activation(
                        out1[:, s * HW + n * NW : s * HW + (n + 1) * NW],
                        ps[:CMID, :],
                        AF.Relu,
                        bias=b1_sb[:, 0:1],
                    )

            # ---- inv_c1 + BN + ReLU -> z [16, (s, hw)] bf16 ---------------
            z_sb = bpool.tile([R, S * HW], dt.bfloat16, tag="z")
            for s in range(S):
                for n in range(NCH):
                    ps = ppool.tile([128, NW], dt.float32, tag="ps")
                    nc.tensor.matmul(
                        ps[:R, :],
                        c1t_sb[:, :],
                        out1[:, s * HW + n * NW : s * HW + (n + 1) * NW],
                        start=True,
                        stop=True,
                    )
                    nc.scalar.activation(
                        z_sb[:, s * HW + n * NW : s * HW + (n + 1) * NW],
                        ps[:R, :],
                        AF.Relu,
                        bias=bi_sb[:, 0:1],
                    )

            # ---- inv_c2 + bias -> w2a [128,(s,hw)], w2b [68,(s,hw)] -------
            w2a = bpool.tile([128, S * HW], dt.bfloat16, tag="w2a")
            w2b = bpool.tile([68, S * HW], dt.bfloat16, tag="w2b")
            for s in range(S):
                for n in range(NCH):
                    sl = slice(s * HW + n * NW, s * HW + (n + 1) * NW)
                    psa = ppool.tile([128, NW], dt.float32, tag="ps")
                    psb = ppool.tile([128, NW], dt.float32, tag="ps")
                    nc.tensor.matmul(
                        psa[:, :],
                        c2t_sb[:, 0:128],
                        z_sb[:, sl],
                        start=True,
                        stop=True,
                    )
                    nc.tensor.matmul(
                        psb[:68, :],
                        c2t_sb[:, 128:196],
                        z_sb[:, sl],
                        start=True,
                        stop=True,
                    )
                    nc.scalar.activation(
                        w2a[:, sl], psa[:, :], AF.Identity, bias=b2ca_sb[:, 0:1]
                    )
                    nc.scalar.activation(
                        w2b[:, sl], psb[:68, :], AF.Identity, bias=b2cb_sb[:, 0:1]
                    )

            # ---- involution operand builds --------------------------------
            # XU: unpadded halo rows, (s,g,m) partition layout
            xu = bpool.tile([NP_INV, XUF], dt.bfloat16, tag="xu")
            xh = bpool.tile([NP_INV, XHF], dt.bfloat16, tag="xh")
            xh2 = bpool.tile([NP_INV, XHF], dt.bfloat16, tag="xh2")
            w2t = bpool.tile([NP_INV, W2F], dt.bfloat16, tag="w2t")
            nc.any.memset(xu[:, :], 0.0)
            nc.any.memset(xh[:, :], 0.0)
            nc.any.memset(xh2[:, :], 0.0)

            # SBUF->SBUF DMAs cannot transpose the partition dim, so bounce
            # the layout changes through DRAM staging tiles (dep-tracked).
            out1d = dpool.tile([CMID, S * HW], dt.bfloat16, tag="out1d")
            nc.sync.dma_start(out=out1d[:, :], in_=out1[:, :])
            xu_ap = xu[:, :]
            o1d_ap = out1d[:, :]
            P_XU = xu_ap.ap[0][0]
            D_O1 = S * HW
            for s in range(S):
                for g in range(G):
                    pb = (s * G + g) * M
                    cb = g * GC
                    # middle chunks m=1..12: all 10 halo rows valid.
                    # iteration (m, c, run): dst partition dim first.
                    nc.sync.dma_start(
                        out=_ap(
                            xu_ap,
                            (pb + 1) * P_XU,
                            [(P_XU, 12), (HR * W, GC), (1, HR * W)],
                        ),
                        in_=_ap(
                            o1d_ap,
                            cb * D_O1 + s * HW + 1 * W,
                            [(RH * W, 12), (D_O1, GC), (1, HR * W)],
                        ),
                    )
                    # m=0: rows 0..6 -> r=3..9
                    nc.sync.dma_start(
                        out=_ap(
                            xu_ap,
                            pb * P_XU + 3 * W,
                            [(P_XU, 1), (HR * W, GC), (1, 7 * W)],
                        ),
                        in_=_ap(
                            o1d_ap,
                            cb * D_O1 + s * HW,
                            [(RH * W, 1), (D_O1, GC), (1, 7 * W)],
                        ),
                    )
                    # m=13: rows 49..55 -> r=0..6
                    nc.sync.dma_start(
                        out=_ap(
                            xu_ap,
                            (pb + 13) * P_XU,
                            [(P_XU, 1), (HR * W, GC), (1, 7 * W)],
                        ),
                        in_=_ap(
                            o1d_ap,
                            cb * D_O1 + s * HW + 49 * W,
                            [(RH * W, 1), (D_O1, GC), (1, 7 * W)],
                        ),
                    )

            # expand XU (56-wide rows) into XH (62-wide zero-padded rows) and
            # XH2 (same, shifted right one element for 4B-aligned odd taps)
            xu_v = xu[:, :].rearrange("p (c r w) -> p c r w", r=HR, w=W)
            xh_v = xh[:, :].rearrange("p (c r w) -> p c r w", r=HR, w=WP)
            xh2_v = xh2[:, :].rearrange("p (c r w) -> p c r w", r=HR, w=WP)
            nc.vector.tensor_copy(xh_v[:, :, :, 3 : 3 + W], xu_v)
            nc.vector.tensor_copy(xh2_v[:, :, :, 4 : 4 + W], xu_v)

            # W2: dynamic weights in (s,g,m) layout, free = (k, rh, w);
            # staged via DRAM (w2d) to transpose the partition dim.
            w2d = dpool.tile([G * KK, S * HW], dt.bfloat16, tag="w2d")
            nc.sync.dma_start(out=w2d[0:128, :], in_=w2a[:, :])
            nc.sync.dma_start(out=w2d[128 : G * KK, :], in_=w2b[:, :])
            w2t_ap = w2t[:, :]
            w2d_ap = w2d[:, :]
            P_W2 = w2t_ap.ap[0][0]
            D_W2 = S * HW
            for s in range(S):
                for g in range(G):
                    pb = (s * G + g) * M
                    # iteration (m, k, run): dst partition dim first
                    nc.sync.dma_start(
                        out=_ap(
                            w2t_ap,
                            pb * P_W2,
                            [(P_W2, M), (RH * W, KK), (1, RH * W)],
                        ),
                        in_=_ap(
                            w2d_ap,
                            (g * KK) * D_W2 + s * HW,
                            [(RH * W, M), (D_W2, KK), (1, RH * W)],
                        ),
                    )

            # ---- involution: 49 taps of mul + accumulate on DVE -----------
            acc = bpool.tile([NP_INV, ACCF], dt.bfloat16, tag="acc")
            tmp = bpool.tile([NP_INV, ACCF], dt.bfloat16, tag="tmp")
            acc_v = acc[:, :].rearrange("p (c r w) -> p c r w", r=RH, w=W)
            tmp_v = tmp[:, :].rearrange("p (c r w) -> p c r w", r=RH, w=W)
            w2t_v = w2t[:, :].rearrange("p (k r w) -> p k r w", k=KK, r=RH)
            for k in range(KK):
                kh, kw = divmod(k, KS)
                if kw % 2 == 0:
                    src_v = xh_v          # offset kh*62+kw even -> 4B aligned
                    wc = kw
                else:
                    src_v = xh2_v         # XH2[w'] = XH[w'-1]; kw+1 even
                    wc = kw + 1
                in0 = src_v[:, :, kh : kh + RH, wc : wc + W]
                in1 = w2t_v[:, k : k + 1, :, :].to_broadcast([NP_INV, GC, RH, W])
                if k == 0:
                    nc.vector.tensor_mul(acc_v, in0, in1)
                else:
                    nc.vector.tensor_mul(tmp_v, in0, in1)
                    nc.vector.tensor_add(acc_v, acc_v, tmp_v)

            # ---- ACC -> (DRAM) -> out2 [64, (s, hw)], BN2+ReLU ------------
            accd = dpool.tile([NP_INV, ACCF], dt.bfloat16, tag="accd")
            nc.sync.dma_start(out=accd[:, :], in_=acc[:, :])
            out2 = bpool.tile([CMID, S * HW], dt.bfloat16, tag="out2")
            o2_ap = out2[:, :]
            P_O2 = o2_ap.ap[0][0]
            acd_ap = accd[:, :]
            for s in range(S):
                for g in range(G):
                    pb = (s * G + g) * M
                    # iteration (c, m, run): dst partition dim first
                    nc.sync.dma_start(
                        out=_ap(
                            o2_ap,
                            (g * GC) * P_O2 + s * HW,
                            [(P_O2, GC), (RH * W, M), (1, RH * W)],
                        ),
                        in_=_ap(
                            acd_ap,
                            pb * ACCF,
                            [(RH * W, GC), (ACCF, M), (1, RH * W)],
                        ),
                    )
            relu2 = bpool.tile([CMID, S * HW], dt.bfloat16, tag="relu2")
            for s in range(S):
                nc.scalar.activation(
                    relu2[:, s * HW : (s + 1) * HW],
                    out2[:, s * HW : (s + 1) * HW],
                    AF.Relu,
                    bias=b2v_sb[:, 0:1],
                    scale=s2v_sb[:, 0:1],
                )

            # ---- conv3 + BN3 + residual + ReLU -> out ---------------------
            w3t_v = w3t_sb[:, :].rearrange("p (k c) -> p k c", k=2)
            for s in range(S):
                for oc in range(2):
                    for n in range(NCH):
                        ps = ppool.tile([128, NW], dt.float32, tag="ps")
                        nc.tensor.matmul(
                            ps[:, :],
                            w3t_v[:, oc, :],
                            relu2[:, s * HW + n * NW : s * HW + (n + 1) * NW],
                            start=True,
                            stop=False,
                        )
                        nc.tensor.matmul(
                            ps[:, :],
                            id_sb[:, :],
                            xbf_v[:, s, oc, n * NW : (n + 1) * NW],
                            start=False,
                            stop=True,
                        )
                        ob = spool.tile([128, NW], dt.float32, tag="obuf")
                        nc.scalar.activation(
                            ob[:, :], ps[:, :], AF.Relu, bias=b3_sb[:, oc : oc + 1]
                        )
                        nc.sync.dma_start(
                            out=out.ap()[
                                s, oc * 128 : (oc + 1) * 128, n * NW : (n + 1) * NW
                            ],
                            in_=ob[:, :],
                        )

    nc.compile()
    _CACHE["nc"] = nc
    return nc


def _f32(a):
    return np.ascontiguousarray(a, dtype=np.float32)


def prep_weights(inputs):
    """Host-side folding of BN scales into conv weights; bf16 casts."""
    f = inputs
    s1 = f["bn1_g"] / np.sqrt(f["bn1_v"] + EPS)
    b1_eff = f["bn1_b"] - f["bn1_m"] * s1
    w1t_eff = (_f32(f["conv1_w"]) * s1[:, None]).T          # [256, 64]

    si = f["inv_bn_g"] / np.sqrt(f["inv_bn_v"] + EPS)
    bi_eff = f["inv_bn_b"] - f["inv_bn_m"] * si
    c1t_eff = (_f32(f["inv_c1_w"]) * si[:, None]).T         # [64, 16]

    c2t_eff = _f32(f["inv_c2_w"]).T                         # [16, 196]
    b2c = _f32(f["inv_c2_b"])

    s2 = f["bn2_g"] / np.sqrt(f["bn2_v"] + EPS)
    b2n = f["bn2_b"] - f["bn2_m"] * s2

    s3 = f["bn3_g"] / np.sqrt(f["bn3_v"] + EPS)
    b3_eff = f["bn3_b"] - f["bn3_m"] * s3
    w3t_eff = (_f32(f["conv3_w"]) * s3[:, None]).T          # [64, 256]

    d = {}
    d["w1t"] = np.ascontiguousarray(
        w1t_eff.reshape(2, 128, CMID).astype(BF16)
    )
    d["b1"] = _f32(b1_eff)[:, None]
    d["c1t"] = np.ascontiguousarray(c1t_eff.astype(BF16))
    d["bi"] = _f32(bi_eff)[:, None]
    d["c2t"] = np.ascontiguousarray(c2t_eff.astype(BF16))
    d["b2ca"] = _f32(b2c[0:128])[:, None]
    d["b2cb"] = _f32(b2c[128:196])[:, None]
    d["s2v"] = _f32(s2)[:, None]
    d["b2v"] = _f32(b2n)[:, None]
    d["w3t"] = np.ascontiguousarray(
        w3t_eff.reshape(CMID, 2, 128).transpose(1, 0, 2).astype(BF16)
    )
    d["b3"] = _f32(b3_eff.reshape(2, 128).T)
    d["ident"] = np.ascontiguousarray(np.eye(128, dtype=np.float32).astype(BF16))
    return d


def make_in_maps(inputs):
    prep = prep_weights(inputs)
    x = _f32(inputs["x"]).reshape(16, CIN, HW)
    in_maps = []
    for i in range(N_CORES):
        m = dict(prep)
        m["xin"] = np.ascontiguousarray(x[S * i : S * i + S])
        in_maps.append(m)
    return in_maps


def kernel(**inputs):
    from concourse.bass_utils import run_bass_kernel_spmd

    nc = build_module()
    in_maps = make_in_maps(inputs)
    res = run_bass_kernel_spmd(nc, in_maps, core_ids=list(range(N_CORES)))
    outs = [res.results[i]["out"].reshape(S, CIN, H, W) for i in range(N_CORES)]
    return np.concatenate(outs, axis=0).astype(np.float32)



# revision 4
# speedup vs baseline: 1.1628x; 1.1628x over previous
"""Trainium2 Bass kernel for nn_Bottleneck_5669356834470 (ResNet bottleneck
with an involution middle layer).

Sharding: data-parallel over batch. 16 samples / 8 cores = 2 samples/core.
All weights replicated (tiny).

Per-core pipeline (spatial 56x56 = 3136 flattened, S=2 samples):
  conv1 (1x1, 256->64) +BN1+ReLU   : PE matmuls (bf16), ACT evac with fused
      scale(folded)+bias+relu, written into zero-padded 62-wide row planes
      (out1p) so the involution halo gather needs no edge special-casing.
  inv_c1 (1x1, 64->16) +BN+ReLU    : PE, ACT evac -> z
  inv_c2 (1x1, 16->196) + bias     : PE (two 98-wide halves, group-aligned),
      ACT evac -> dynamic weights w2a/w2b
  involution (G=4, 7x7 dynamic)    : DVE computes the 49 per-tap products
      (tensor_tensor mult, bf16 2x mode) in a (sample, group, 4-row-chunk)
      partition layout (112 partitions); the 48 accumulation adds run on the
      otherwise-idle Tensor engine as identity matmuls accumulating into a
      7-bank fp32 PSUM region. Dynamic weights broadcast across the 16 group
      channels via a 0-stride AP dim; spatial shifts are free-dim slices of
      the zero-padded halo tensor xh (xh2 = xh shifted one element for
      4B-aligned odd taps).
  BN2+ReLU                         : ACT (after corner-turn back to channel
      partitions)
  conv3 (1x1, 64->256) +BN3 + residual + ReLU : PE (residual folded in as an
      identity matmul over bf16 input), ACT evac w/ fused bias+relu.

Corner-turns (partition<->free exchanges) bounce through DRAM staging tiles;
SBUF APs can only carry the partition dim as their leading dim. PSUM: one
[128, 3584] f32 tile (7 banks); conv phases cycle chunk slots through it,
the involution uses it as the accumulator.

Compute dtype bf16 (f32 PSUM accumulation); output f32.
"""

import sys

sys.path.insert(0, "/opt/trn_rl_repo")

import numpy as np
import ml_dtypes

BF16 = ml_dtypes.bfloat16

S = 2            # samples per core
N_CORES = 8
CIN = 256
CMID = 64
G = 4            # involution groups
GC = 16          # channels per group
KS = 7           # involution kernel size
KK = KS * KS     # 49
R = 16           # dyn-weight bottleneck channels
H = W = 56
HW = H * W       # 3136
NCH = 7          # spatial chunks for matmul N dim (448 positions = 8 rows)
NW = HW // NCH   # 448
M = 14           # 4-row chunks per (sample, group)
RH = 4           # output rows per chunk
HR = 10          # halo rows stored per chunk (-3..+6)
WP = 62          # padded row width
PR = 65          # padded rows per plane (-3..61)
PLANE = PR * WP  # 4030 elems per (sample, channel) plane
NP_INV = S * G * M          # 112 involution partitions
XHF = GC * HR * WP          # 9920 free elems per XH partition
W2F = KK * RH * W           # 10976 free elems per W2 partition
ACCF = GC * RH * W          # 3584 acc free elems per partition
NBANK = 7                   # psum bank-chunks (512 f32 each)
EPS = 1e-5

_CACHE = {}


def _ap(tile_ap, off, dims):
    """Raw strided AP on a tile's underlying tensor. dims=[(step,count),...]
    in elements; for SBUF dims[0] must be the partition dim (step = the
    tile AP's leading stride)."""
    import bass_rust

    return bass_rust.AP(tile_ap.tensor, tile_ap.offset + off, [list(d) for d in dims])


def build_module():
    if "nc" in _CACHE:
        return _CACHE["nc"]
    import concourse.bacc as bacc
    import concourse.mybir as mybir
    import concourse.tile as tile

    dt = mybir.dt
    AF = mybir.ActivationFunctionType

    nc = bacc.Bacc("TRN2", debug=False, num_devices=N_CORES)

    # ---- DRAM I/O ----------------------------------------------------------
    xin = nc.dram_tensor("xin", [S, CIN, HW], dt.float32, kind="ExternalInput")
    w1t = nc.dram_tensor("w1t", [2, 128, CMID], dt.bfloat16, kind="ExternalInput")
    b1 = nc.dram_tensor("b1", [CMID, 1], dt.float32, kind="ExternalInput")
    c1t = nc.dram_tensor("c1t", [CMID, R], dt.bfloat16, kind="ExternalInput")
    bi = nc.dram_tensor("bi", [R, 1], dt.float32, kind="ExternalInput")
    c2ta = nc.dram_tensor("c2ta", [R, 98], dt.bfloat16, kind="ExternalInput")
    c2tb = nc.dram_tensor("c2tb", [R, 98], dt.bfloat16, kind="ExternalInput")
    b2a = nc.dram_tensor("b2a", [98, 1], dt.float32, kind="ExternalInput")
    b2b = nc.dram_tensor("b2b", [98, 1], dt.float32, kind="ExternalInput")
    s2v = nc.dram_tensor("s2v", [CMID, 1], dt.float32, kind="ExternalInput")
    b2v = nc.dram_tensor("b2v", [CMID, 1], dt.float32, kind="ExternalInput")
    w3t = nc.dram_tensor("w3t", [2, CMID, 128], dt.bfloat16, kind="ExternalInput")
    b3 = nc.dram_tensor("b3", [128, 2], dt.float32, kind="ExternalInput")
    ident = nc.dram_tensor("ident", [128, 128], dt.bfloat16, kind="ExternalInput")
    out = nc.dram_tensor("out", [S, CIN, HW], dt.float32, kind="ExternalOutput")

    with tile.TileContext(nc) as tc:
        with (
            tc.tile_pool(name="consts", bufs=1) as cpool,
            tc.tile_pool(name="big", bufs=1) as bpool,
            tc.tile_pool(name="tmp", bufs=3) as tpool,
            tc.tile_pool(name="psum", bufs=1, space="PSUM") as ppool,
            tc.tile_pool(name="stage", bufs=4) as spool,
            tc.tile_pool(name="dstage", bufs=1, space="DRAM") as dpool,
        ):
            # ---- constants -> SBUF ----------------------------------------
            w1t_sb = cpool.tile([128, 2 * CMID], dt.bfloat16, tag="w1t")
            nc.sync.dma_start(
                out=w1t_sb[:, :].rearrange("p (k c) -> p k c", k=2),
                in_=w1t.ap().rearrange("k p c -> p k c"),
            )
            b1_sb = cpool.tile([CMID, 1], dt.float32, tag="b1")
            nc.sync.dma_start(out=b1_sb[:, :], in_=b1.ap())
            c1t_sb = cpool.tile([CMID, R], dt.bfloat16, tag="c1t")
            nc.sync.dma_start(out=c1t_sb[:, :], in_=c1t.ap())
            bi_sb = cpool.tile([R, 1], dt.float32, tag="bi")
            nc.sync.dma_start(out=bi_sb[:, :], in_=bi.ap())
            c2ta_sb = cpool.tile([R, 98], dt.bfloat16, tag="c2ta")
            nc.sync.dma_start(out=c2ta_sb[:, :], in_=c2ta.ap())
            c2tb_sb = cpool.tile([R, 98], dt.bfloat16, tag="c2tb")
            nc.sync.dma_start(out=c2tb_sb[:, :], in_=c2tb.ap())
            b2a_sb = cpool.tile([98, 1], dt.float32, tag="b2a")
            nc.sync.dma_start(out=b2a_sb[:, :], in_=b2a.ap())
            b2b_sb = cpool.tile([98, 1], dt.float32, tag="b2b")
            nc.sync.dma_start(out=b2b_sb[:, :], in_=b2b.ap())
            s2v_sb = cpool.tile([CMID, 1], dt.float32, tag="s2v")
            nc.sync.dma_start(out=s2v_sb[:, :], in_=s2v.ap())
            b2v_sb = cpool.tile([CMID, 1], dt.float32, tag="b2v")
            nc.sync.dma_start(out=b2v_sb[:, :], in_=b2v.ap())
            w3t_sb = cpool.tile([CMID, 2 * 128], dt.bfloat16, tag="w3t")
            nc.sync.dma_start(
                out=w3t_sb[:, :].rearrange("p (k c) -> p k c", k=2),
                in_=w3t.ap().rearrange("k p c -> p k c"),
            )
            b3_sb = cpool.tile([128, 2], dt.float32, tag="b3")
            nc.sync.dma_start(out=b3_sb[:, :], in_=b3.ap())
            id_sb = cpool.tile([128, 128], dt.bfloat16, tag="ident")
            nc.sync.dma_start(out=id_sb[:, :], in_=ident.ap())

            # ---- big SBUF tensors -----------------------------------------
            xbf = bpool.tile([128, S * 2 * HW], dt.bfloat16, tag="xbf")
            xbf_v = xbf[:, :].rearrange("p (s k f) -> p s k f", s=S, k=2)
            out1p = bpool.tile([CMID, S * PLANE], dt.bfloat16, tag="out1p")
            o1p_v = out1p[:, :].rearrange("p (s r w) -> p s r w", s=S, w=WP)
            z_sb = bpool.tile([R, S * HW], dt.bfloat16, tag="z")
            w2a = bpool.tile([98, S * HW], dt.bfloat16, tag="w2a")
            w2b = bpool.tile([98, S * HW], dt.bfloat16, tag="w2b")
            xh = bpool.tile([NP_INV, XHF], dt.bfloat16, tag="xh")
            xh2 = bpool.tile([NP_INV, XHF], dt.bfloat16, tag="xh2")
            w2t = bpool.tile([NP_INV, W2F], dt.bfloat16, tag="w2t")
            accsb = bpool.tile([NP_INV, ACCF], dt.bfloat16, tag="accsb")
            out2 = bpool.tile([CMID, S * HW], dt.bfloat16, tag="out2")
            relu2 = bpool.tile([CMID, S * HW], dt.bfloat16, tag="relu2")

            # one 7-bank PSUM tile: conv chunk slots + involution accumulator
            acc = ppool.tile([128, NBANK * 512], dt.float32, tag="acc")
            slot_ctr = [0]

            def next_slot():
                s = slot_ctr[0]
                slot_ctr[0] = (s + 1) % NBANK
                return s

            # zero out1p pads once (DVE idle at kernel start; everything else
            # overwrites its region)
            nc.vector.memset(out1p[:, :], 0.0)

            # ---- x load (f32 -> bf16 cast during SWDGE DMA), per sample ---
            for s in range(S):
                nc.gpsimd.dma_start(
                    out=xbf_v[:, s],
                    in_=xin.ap()[s].rearrange("(k p) f -> p k f", p=128),
                )

            # ---- conv1 + BN1 + ReLU -> out1p (padded planes) --------------
            w1t_v = w1t_sb[:, :].rearrange("p (k c) -> p k c", k=2)
            for s in range(S):
                for n in range(NCH):
                    sl = next_slot() * 512
                    ps = acc[:CMID, sl : sl + NW]
                    for kc in range(2):
                        nc.tensor.matmul(
                            ps,
                            w1t_v[:, kc, :],
                            xbf_v[:, s, kc, n * NW : (n + 1) * NW],
                            start=(kc == 0),
                            stop=(kc == 1),
                        )
                    nc.scalar.activation(
                        o1p_v[:, s, 3 + 8 * n : 3 + 8 * n + 8, 3 : 3 + W],
                        ps.rearrange("p (r w) -> p r w", r=8),
                        AF.Relu,
                        bias=b1_sb[:, 0:1],
                    )

            # ---- inv_c1 + BN + ReLU -> z [16, (s, hw)] --------------------
            for s in range(S):
                for n in range(NCH):
                    sl = next_slot() * 512
                    ps = acc[:R, sl : sl + NW]
                    nc.tensor.matmul(
                        ps,
                        c1t_sb[:, :],
                        o1p_v[:, s, 3 + 8 * n : 3 + 8 * n + 8, 3 : 3 + W],
                        start=True,
                        stop=True,
                    )
                    nc.scalar.activation(
                        z_sb[:, s * HW + n * NW : s * HW + (n + 1) * NW],
                        ps,
                        AF.Relu,
                        bias=bi_sb[:, 0:1],
                    )

            # ---- inv_c2 + bias -> w2a/w2b [98, (s,hw)] (group-aligned) ----
            for s in range(S):
                for n in range(NCH):
                    zsl = z_sb[:, s * HW + n * NW : s * HW + (n + 1) * NW]
                    sla = next_slot() * 512
                    slb = next_slot() * 512
                    psa = acc[:98, sla : sla + NW]
                    psb = acc[:98, slb : slb + NW]
                    nc.tensor.matmul(psa, c2ta_sb[:, :], zsl, start=True, stop=True)
                    nc.tensor.matmul(psb, c2tb_sb[:, :], zsl, start=True, stop=True)
                    osl = slice(s * HW + n * NW, s * HW + (n + 1) * NW)
                    nc.scalar.activation(
                        w2a[:, osl], psa, AF.Identity, bias=b2a_sb[:, 0:1]
                    )
                    nc.scalar.activation(
                        w2b[:, osl], psb, AF.Identity, bias=b2b_sb[:, 0:1]
                    )

            # ---- corner-turns through DRAM --------------------------------
            # out1p -> o1d [s, c, plane]
            o1d = dpool.tile([S * CMID, PLANE], dt.bfloat16, tag="o1d")
            o1d_ap = o1d[:, :]
            o1p_ap = out1p[:, :]
            P_O1P = o1p_ap.ap[0][0]
            nc.sync.dma_start(
                out=_ap(o1d_ap, 0, [(PLANE, CMID), (CMID * PLANE, S), (1, PLANE)]),
                in_=_ap(o1p_ap, 0, [(P_O1P, CMID), (PLANE, S), (1, PLANE)]),
            )
            # xh gather: per (s,g) all 14 chunks uniformly (pads come along)
            xh_ap = xh[:, :]
            P_XH = xh_ap.ap[0][0]
            for s in range(S):
                for g in range(G):
                    pb = (s * G + g) * M
                    nc.sync.dma_start(
                        out=_ap(
                            xh_ap,
                            pb * P_XH,
                            [(P_XH, M), (HR * WP, GC), (1, HR * WP)],
                        ),
                        in_=_ap(
                            o1d_ap,
                            s * CMID * PLANE + (g * GC) * PLANE,
                            [(RH * WP, M), (PLANE, GC), (1, HR * WP)],
                        ),
                    )
            # xh2 = xh shifted right one element (partition-preserving DMA;
            # xh2[:, 0] is never read)
            xh2_ap = xh2[:, :]
            P_XH2 = xh2_ap.ap[0][0]
            nc.scalar.dma_start(
                out=_ap(xh2_ap, 1, [(P_XH2, NP_INV), (1, XHF - 1)]),
                in_=_ap(xh_ap, 0, [(P_XH, NP_INV), (1, XHF - 1)]),
            )

            # w2 -> w2d [s, ko, hw] then gather -> w2t [(s,g,m), (k, r, w)]
            w2d = dpool.tile([S * 2 * 98, HW], dt.bfloat16, tag="w2d")
            w2d_ap = w2d[:, :]
            for half, wsrc in ((0, w2a), (1, w2b)):
                wsrc_ap = wsrc[:, :]
                P_WS = wsrc_ap.ap[0][0]
                nc.sync.dma_start(
                    out=_ap(
                        w2d_ap,
                        half * 98 * HW,
                        [(HW, 98), (2 * 98 * HW, S), (1, HW)],
                    ),
                    in_=_ap(wsrc_ap, 0, [(P_WS, 98), (HW, S), (1, HW)]),
                )
            w2t_ap = w2t[:, :]
            P_W2T = w2t_ap.ap[0][0]
            for s in range(S):
                for g in range(G):
                    pb = (s * G + g) * M
                    nc.sync.dma_start(
                        out=_ap(
                            w2t_ap,
                            pb * P_W2T,
                            [(P_W2T, M), (RH * W, KK), (1, RH * W)],
                        ),
                        in_=_ap(
                            w2d_ap,
                            s * 2 * 98 * HW + g * KK * HW,
                            [(RH * W, M), (HW, KK), (1, RH * W)],
                        ),
                    )

            # ---- involution: DVE products + PE identity-matmul accumulate -
            xh_v = xh[:, :].rearrange("p (c r w) -> p c r w", r=HR, w=WP)
            xh2_v = xh2[:, :].rearrange("p (c r w) -> p c r w", r=HR, w=WP)
            w2t_v = w2t[:, :].rearrange("p (k r w) -> p k r w", k=KK, r=RH)
            # even-kw taps first: they only need xh, so the DVE can start
            # before the xh2 shift-copy lands
            taps = [k for k in range(KK) if (k % KS) % 2 == 0] + [
                k for k in range(KK) if (k % KS) % 2 == 1
            ]
            for i, k in enumerate(taps):
                kh, kw = divmod(k, KS)
                if kw % 2 == 0:
                    src_v, wc = xh_v, kw
                else:
                    src_v, wc = xh2_v, kw + 1
                tmp = tpool.tile([NP_INV, ACCF], dt.bfloat16, tag="tmp")
                tmp_v = tmp[:, :].rearrange("p (c r w) -> p c r w", r=RH, w=W)
                nc.vector.tensor_mul(
                    tmp_v,
                    src_v[:, :, kh : kh + RH, wc : wc + W],
                    w2t_v[:, k : k + 1, :, :].to_broadcast([NP_INV, GC, RH, W]),
                )
                for n in range(NBANK):
                    nc.tensor.matmul(
                        acc[:NP_INV, n * 512 : (n + 1) * 512],
                        id_sb[:NP_INV, :NP_INV],
                        tmp[:, n * 512 : (n + 1) * 512],
                        start=(i == 0),
                        stop=(i == KK - 1),
                        skip_group_check=True,
                    )

            # ---- evac accumulator, corner-turn back, BN2+ReLU -------------
            nc.scalar.activation(accsb[:, :], acc[:NP_INV, :ACCF], AF.Identity)
            accd = dpool.tile([NP_INV, ACCF], dt.bfloat16, tag="accd")
            accd_ap = accd[:, :]
            accsb_ap = accsb[:, :]
            P_ACC = accsb_ap.ap[0][0]
            nc.sync.dma_start(out=accd_ap, in_=accsb_ap)
            out2_ap = out2[:, :]
            P_O2 = out2_ap.ap[0][0]
            for s in range(S):
                for g in range(G):
                    pb = (s * G + g) * M
                    nc.sync.dma_start(
                        out=_ap(
                            out2_ap,
                            (g * GC) * P_O2 + s * HW,
                            [(P_O2, GC), (RH * W, M), (1, RH * W)],
                        ),
                        in_=_ap(
                            accd_ap,
                            pb * ACCF,
                            [(RH * W, GC), (ACCF, M), (1, RH * W)],
                        ),
                    )
            nc.scalar.activation(
                relu2[:, :],
                out2[:, :],
                AF.Relu,
                bias=b2v_sb[:, 0:1],
                scale=s2v_sb[:, 0:1],
            )

            # ---- conv3 + BN3 + residual + ReLU -> out ---------------------
            w3t_v = w3t_sb[:, :].rearrange("p (k c) -> p k c", k=2)
            for s in range(S):
                for oc in range(2):
                    for n in range(NCH):
                        sl = next_slot() * 512
                        ps = acc[:, sl : sl + NW]
                        nc.tensor.matmul(
                            ps,
                            w3t_v[:, oc, :],
                            relu2[:, s * HW + n * NW : s * HW + (n + 1) * NW],
                            start=True,
                            stop=False,
                        )
                        nc.tensor.matmul(
                            ps,
                            id_sb[:, :],
                            xbf_v[:, s, oc, n * NW : (n + 1) * NW],
                            start=False,
                            stop=True,
                        )
                        ob = spool.tile([128, NW], dt.float32, tag="obuf")
                        nc.scalar.activation(
                            ob[:, :], ps, AF.Relu, bias=b3_sb[:, oc : oc + 1]
                        )
                        nc.sync.dma_start(
                            out=out.ap()[
                                s, oc * 128 : (oc + 1) * 128, n * NW : (n + 1) * NW
                            ],
                            in_=ob[:, :],
                        )

    nc.compile()
    _CACHE["nc"] = nc
    return nc


def _f32(a):
    return np.ascontiguousarray(a, dtype=np.float32)


def prep_weights(inputs):
    """Host-side folding of BN scales into conv weights; bf16 casts."""
    f = inputs
    s1 = f["bn1_g"] / np.sqrt(f["bn1_v"] + EPS)
    b1_eff = f["bn1_b"] - f["bn1_m"] * s1
    w1t_eff = (_f32(f["conv1_w"]) * s1[:, None]).T          # [256, 64]

    si = f["inv_bn_g"] / np.sqrt(f["inv_bn_v"] + EPS)
    bi_eff = f["inv_bn_b"] - f["inv_bn_m"] * si
    c1t_eff = (_f32(f["inv_c1_w"]) * si[:, None]).T         # [64, 16]

    c2t_eff = _f32(f["inv_c2_w"]).T                         # [16, 196]
    b2c = _f32(f["inv_c2_b"])

    s2 = f["bn2_g"] / np.sqrt(f["bn2_v"] + EPS)
    b2n = f["bn2_b"] - f["bn2_m"] * s2

    s3 = f["bn3_g"] / np.sqrt(f["bn3_v"] + EPS)
    b3_eff = f["bn3_b"] - f["bn3_m"] * s3
    w3t_eff = (_f32(f["conv3_w"]) * s3[:, None]).T          # [64, 256]

    d = {}
    d["w1t"] = np.ascontiguousarray(
        w1t_eff.reshape(2, 128, CMID).astype(BF16)
    )
    d["b1"] = _f32(b1_eff)[:, None]
    d["c1t"] = np.ascontiguousarray(c1t_eff.astype(BF16))
    d["bi"] = _f32(bi_eff)[:, None]
    d["c2ta"] = np.ascontiguousarray(c2t_eff[:, 0:98].astype(BF16))
    d["c2tb"] = np.ascontiguousarray(c2t_eff[:, 98:196].astype(BF16))
    d["b2a"] = _f32(b2c[0:98])[:, None]
    d["b2b"] = _f32(b2c[98:196])[:, None]
    d["s2v"] = _f32(s2)[:, None]
    d["b2v"] = _f32(b2n)[:, None]
    d["w3t"] = np.ascontiguousarray(
        w3t_eff.reshape(CMID, 2, 128).transpose(1, 0, 2).astype(BF16)
    )
    d["b3"] = _f32(b3_eff.reshape(2, 128).T)
    d["ident"] = np.ascontiguousarray(np.eye(128, dtype=np.float32).astype(BF16))
    return d


def make_in_maps(inputs):
    prep = prep_weights(inputs)
    x = _f32(inputs["x"]).reshape(16, CIN, HW)
    in_maps = []
    for i in range(N_CORES):
        m = dict(prep)
        m["xin"] = np.ascontiguousarray(x[S * i : S * i + S])
        in_maps.append(m)
    return in_maps


def kernel(**inputs):
    from concourse.bass_utils import run_bass_kernel_spmd

    nc = build_module()
    in_maps = make_in_maps(inputs)
    res = run_bass_kernel_spmd(nc, in_maps, core_ids=list(range(N_CORES)))
    outs = [res.results[i]["out"].reshape(S, CIN, H, W) for i in range(N_CORES)]
    return np.concatenate(outs, axis=0).astype(np.float32)


# revision 10
# speedup vs baseline: 1.4369x; 1.2357x over previous
"""Trainium2 Bass kernel for nn_Bottleneck_5669356834470 (ResNet bottleneck
with an involution middle layer).

Sharding: data-parallel over batch. 16 samples / 8 cores = 2 samples/core.
All weights replicated (tiny).

Per-core pipeline (spatial 56x56 = 3136 flattened, S=2 samples):
  conv1 (1x1, 256->64) +BN1+ReLU   : PE matmuls (bf16), ACT evac with fused
      scale(folded)+bias+relu, written into zero-padded 62-wide row planes
      (out1p) so the involution halo gather needs no edge special-casing.
  inv_c1 (1x1, 64->16) +BN+ReLU    : PE, ACT evac -> z
  inv_c2 (1x1, 16->196) + bias     : PE (two 98-wide halves, group-aligned),
      ACT evac -> dynamic weights w2a/w2b
  involution (G=4, 7x7 dynamic)    : DVE computes the 49 per-tap products
      (tensor_tensor mult, bf16 2x mode) in a (sample, group, 4-row-chunk)
      partition layout (112 partitions); the 48 accumulation adds run on the
      otherwise-idle Tensor engine as identity matmuls accumulating into a
      7-bank fp32 PSUM region. BN2's bias (pre-divided by its scale) is added
      by one extra K=1 matmul group; its scale is folded into conv3's weights
      host-side, so the PSUM evacuation IS the BN2+ReLU.
  conv3 (1x1, 64->256) +BN3 + residual + ReLU : PE (residual folded in as an
      identity matmul over bf16 input), ACT evac w/ fused bias+relu.

Corner-turns (partition<->free exchanges) bounce through DRAM staging tiles
(SBUF APs can only carry the partition dim as their leading dim); the xh path
rides the scalar HWDGE ring, the w2 path the sync ring, so they overlap.
PSUM pools are opened sequentially (convs / 7-bank involution accumulator /
conv3) so conv chunks pipeline instead of serializing on whole-tile WARs.

Compute dtype bf16 (f32 PSUM accumulation); output f32.
"""

import sys

sys.path.insert(0, "/opt/trn_rl_repo")

import numpy as np
import ml_dtypes

BF16 = ml_dtypes.bfloat16

S = 2            # samples per core
N_CORES = 8
CIN = 256
CMID = 64
G = 4            # involution groups
GC = 16          # channels per group
KS = 7           # involution kernel size
KK = KS * KS     # 49
R = 16           # dyn-weight bottleneck channels
H = W = 56
HW = H * W       # 3136
NCH = 7          # spatial chunks for matmul N dim (448 positions = 8 rows)
NW = HW // NCH   # 448
M = 14           # 4-row chunks per (sample, group)
RH = 4           # output rows per chunk
HR = 10          # halo rows stored per chunk (-3..+6)
WP = 62          # padded row width
PR = 65          # padded rows per plane (-3..61)
PLANE = PR * WP  # 4030 elems per (sample, channel) plane
NP_INV = S * G * M          # 112 involution partitions
XHF = GC * HR * WP          # 9920 free elems per XH partition
W2F = KK * RH * W           # 10976 free elems per W2 partition
ACCF = GC * RH * W          # 3584 acc free elems per partition
NBANK = 7                   # psum bank-chunks (512 f32 each)
EPS = 1e-5

_CACHE = {}


def _ap(tile_ap, off, dims):
    """Raw strided AP on a tile's underlying tensor. dims=[(step,count),...]
    in elements; for SBUF dims[0] must be the partition dim (step = the
    tile AP's leading stride)."""
    import bass_rust

    return bass_rust.AP(tile_ap.tensor, tile_ap.offset + off, [list(d) for d in dims])


def build_module():
    if "nc" in _CACHE:
        return _CACHE["nc"]
    import concourse.bacc as bacc
    import concourse.mybir as mybir
    import concourse.tile as tile

    dt = mybir.dt
    AF = mybir.ActivationFunctionType

    nc = bacc.Bacc("TRN2", debug=False, num_devices=N_CORES)

    # ---- DRAM I/O ----------------------------------------------------------
    xin = nc.dram_tensor("xin", [S, CIN, HW], dt.float32, kind="ExternalInput")
    w1t = nc.dram_tensor("w1t", [2, 128, CMID], dt.bfloat16, kind="ExternalInput")
    b1 = nc.dram_tensor("b1", [CMID, 1], dt.float32, kind="ExternalInput")
    c1t = nc.dram_tensor("c1t", [CMID, R], dt.bfloat16, kind="ExternalInput")
    bi = nc.dram_tensor("bi", [R, 1], dt.float32, kind="ExternalInput")
    c2ta = nc.dram_tensor("c2ta", [R, 98], dt.bfloat16, kind="ExternalInput")
    c2tb = nc.dram_tensor("c2tb", [R, 98], dt.bfloat16, kind="ExternalInput")
    b2a = nc.dram_tensor("b2a", [98, 1], dt.float32, kind="ExternalInput")
    b2b = nc.dram_tensor("b2b", [98, 1], dt.float32, kind="ExternalInput")
    # BN2 bias patterns (per group, over the involution free dim) and the
    # group-indicator rows that route pattern g to partitions of group g
    b2pat = nc.dram_tensor("b2pat", [G, ACCF], dt.bfloat16, kind="ExternalInput")
    gsel = nc.dram_tensor("gsel", [G, 128], dt.bfloat16, kind="ExternalInput")
    w3t = nc.dram_tensor("w3t", [2, CMID, 128], dt.bfloat16, kind="ExternalInput")
    b3 = nc.dram_tensor("b3", [128, 2], dt.float32, kind="ExternalInput")
    ident = nc.dram_tensor("ident", [128, 128], dt.bfloat16, kind="ExternalInput")
    out = nc.dram_tensor("out", [S, CIN, HW], dt.float32, kind="ExternalOutput")

    with tile.TileContext(nc) as tc:
        with (
            tc.tile_pool(name="consts", bufs=1) as cpool,
            tc.tile_pool(name="big", bufs=1) as bpool,
            tc.tile_pool(name="tmp", bufs=3) as tpool,
            tc.tile_pool(name="stage", bufs=4) as spool,
            tc.tile_pool(name="dstage", bufs=1, space="DRAM") as dpool,
        ):
            # ---- constants -> SBUF ----------------------------------------
            w1t_sb = cpool.tile([128, 2 * CMID], dt.bfloat16, tag="w1t")
            nc.sync.dma_start(
                out=w1t_sb[:, :].rearrange("p (k c) -> p k c", k=2),
                in_=w1t.ap().rearrange("k p c -> p k c"),
            )
            b1_sb = cpool.tile([CMID, 1], dt.float32, tag="b1")
            nc.sync.dma_start(out=b1_sb[:, :], in_=b1.ap())
            c1t_sb = cpool.tile([CMID, R], dt.bfloat16, tag="c1t")
            nc.sync.dma_start(out=c1t_sb[:, :], in_=c1t.ap())
            bi_sb = cpool.tile([R, 1], dt.float32, tag="bi")
            nc.sync.dma_start(out=bi_sb[:, :], in_=bi.ap())
            c2ta_sb = cpool.tile([R, 98], dt.bfloat16, tag="c2ta")
            nc.sync.dma_start(out=c2ta_sb[:, :], in_=c2ta.ap())
            c2tb_sb = cpool.tile([R, 98], dt.bfloat16, tag="c2tb")
            nc.sync.dma_start(out=c2tb_sb[:, :], in_=c2tb.ap())
            b2a_sb = cpool.tile([98, 1], dt.float32, tag="b2a")
            nc.sync.dma_start(out=b2a_sb[:, :], in_=b2a.ap())
            b2b_sb = cpool.tile([98, 1], dt.float32, tag="b2b")
            nc.sync.dma_start(out=b2b_sb[:, :], in_=b2b.ap())
            b2pat_sb = cpool.tile([G, ACCF], dt.bfloat16, tag="b2pat")
            nc.sync.dma_start(out=b2pat_sb[:, :], in_=b2pat.ap())
            gsel_sb = cpool.tile([G, 128], dt.bfloat16, tag="gsel")
            nc.sync.dma_start(out=gsel_sb[:, :], in_=gsel.ap())
            w3t_sb = cpool.tile([CMID, 2 * 128], dt.bfloat16, tag="w3t")
            nc.sync.dma_start(
                out=w3t_sb[:, :].rearrange("p (k c) -> p k c", k=2),
                in_=w3t.ap().rearrange("k p c -> p k c"),
            )
            b3_sb = cpool.tile([128, 2], dt.float32, tag="b3")
            nc.sync.dma_start(out=b3_sb[:, :], in_=b3.ap())
            id_sb = cpool.tile([128, 128], dt.bfloat16, tag="ident")
            nc.sync.dma_start(out=id_sb[:, :], in_=ident.ap())

            # ---- big SBUF tensors -----------------------------------------
            xbf = bpool.tile([128, S * 2 * HW], dt.bfloat16, tag="xbf")
            xbf_v = xbf[:, :].rearrange("p (s k f) -> p s k f", s=S, k=2)
            out1p = bpool.tile([CMID, S * PLANE], dt.bfloat16, tag="out1p")
            o1p_v = out1p[:, :].rearrange("p (s r w) -> p s r w", s=S, w=WP)
            z_sb = bpool.tile([R, S * HW], dt.bfloat16, tag="z")
            w2a = bpool.tile([98, S * HW], dt.bfloat16, tag="w2a")
            w2b = bpool.tile([98, S * HW], dt.bfloat16, tag="w2b")
            xh = bpool.tile([NP_INV, XHF], dt.bfloat16, tag="xh")
            xh2 = bpool.tile([NP_INV, XHF], dt.bfloat16, tag="xh2")
            w2t = bpool.tile([NP_INV, W2F], dt.bfloat16, tag="w2t")
            accsb = bpool.tile([NP_INV, ACCF], dt.bfloat16, tag="accsb")
            out2 = bpool.tile([CMID, S * HW], dt.bfloat16, tag="out2")

            o1d = dpool.tile([S * CMID, PLANE], dt.bfloat16, tag="o1d")
            o1d_ap = o1d[:, :]
            w2d = dpool.tile([S * 2 * 98, HW], dt.bfloat16, tag="w2d")
            w2d_ap = w2d[:, :]
            o1p_ap = out1p[:, :]
            P_O1P = o1p_ap.ap[0][0]

            # zero out1p pads once (DVE idle at kernel start)
            nc.vector.memset(out1p[:, :], 0.0)

            # ---- x load (f32 -> bf16 cast during SWDGE DMA), per sample ---
            for s in range(S):
                nc.gpsimd.dma_start(
                    out=xbf_v[:, s],
                    in_=xin.ap()[s].rearrange("(k p) f -> p k f", p=128),
                )

            w1t_v = w1t_sb[:, :].rearrange("p (k c) -> p k c", k=2)

            # ---- conv chain, per sample; staging DMAs interleaved ---------
            with tc.tile_pool(name="psc", bufs=6, space="PSUM") as pcv:
                for s in range(S):
                    # conv1 + BN1 + ReLU -> out1p (padded planes)
                    for n in range(NCH):
                        ps = pcv.tile([CMID, NW], dt.float32, tag="ps1")
                        for kc in range(2):
                            nc.tensor.matmul(
                                ps[:, :],
                                w1t_v[:, kc, :],
                                xbf_v[:, s, kc, n * NW : (n + 1) * NW],
                                start=(kc == 0),
                                stop=(kc == 1),
                            )
                        nc.scalar.activation(
                            o1p_v[:, s, 3 + 8 * n : 3 + 8 * n + 8, 3 : 3 + W],
                            ps[:, :].rearrange("p (r w) -> p r w", r=8),
                            AF.Relu,
                            bias=b1_sb[:, 0:1],
                        )
                    # out1p(s) -> o1d(s)  [c, plane] (scalar HWDGE ring; its
                    # deps - the evacs just above - are already retired)
                    nc.scalar.dma_start(
                        out=_ap(o1d_ap, s * CMID * PLANE, [(PLANE, CMID), (1, PLANE)]),
                        in_=_ap(o1p_ap, s * PLANE, [(P_O1P, CMID), (1, PLANE)]),
                    )
                    # inv_c1 + BN + ReLU -> z
                    for n in range(NCH):
                        ps = pcv.tile([R, NW], dt.float32, tag="ps1")
                        nc.tensor.matmul(
                            ps[:, :],
                            c1t_sb[:, :],
                            o1p_v[:, s, 3 + 8 * n : 3 + 8 * n + 8, 3 : 3 + W],
                            start=True,
                            stop=True,
                        )
                        nc.scalar.activation(
                            z_sb[:, s * HW + n * NW : s * HW + (n + 1) * NW],
                            ps[:, :],
                            AF.Relu,
                            bias=bi_sb[:, 0:1],
                        )
                    # inv_c2 + bias -> w2a/w2b (group-aligned 98+98)
                    for n in range(NCH):
                        zsl = z_sb[:, s * HW + n * NW : s * HW + (n + 1) * NW]
                        psa = pcv.tile([98, NW], dt.float32, tag="ps1")
                        psb = pcv.tile([98, NW], dt.float32, tag="ps1")
                        nc.tensor.matmul(
                            psa[:, :], c2ta_sb[:, :], zsl, start=True, stop=True
                        )
                        nc.tensor.matmul(
                            psb[:, :], c2tb_sb[:, :], zsl, start=True, stop=True
                        )
                        osl = slice(s * HW + n * NW, s * HW + (n + 1) * NW)
                        nc.scalar.activation(
                            w2a[:, osl], psa[:, :], AF.Identity, bias=b2a_sb[:, 0:1]
                        )
                        nc.scalar.activation(
                            w2b[:, osl], psb[:, :], AF.Identity, bias=b2b_sb[:, 0:1]
                        )
                    # w2(s) -> w2d(s) [ko, hw] (sync ring, runs in parallel)
                    for half, wsrc in ((0, w2a), (1, w2b)):
                        wsrc_ap = wsrc[:, :]
                        P_WS = wsrc_ap.ap[0][0]
                        nc.sync.dma_start(
                            out=_ap(
                                w2d_ap,
                                s * 2 * 98 * HW + half * 98 * HW,
                                [(HW, 98), (1, HW)],
                            ),
                            in_=_ap(wsrc_ap, s * HW, [(P_WS, 98), (1, HW)]),
                        )

                # ---- corner-turn gathers ------------------------------------
                # w2t gather on the sync ring
                w2t_ap = w2t[:, :]
                P_W2T = w2t_ap.ap[0][0]
                for s in range(S):
                    for g in range(G):
                        pb = (s * G + g) * M
                        nc.sync.dma_start(
                            out=_ap(
                                w2t_ap,
                                pb * P_W2T,
                                [(P_W2T, M), (RH * W, KK), (1, RH * W)],
                            ),
                            in_=_ap(
                                w2d_ap,
                                s * 2 * 98 * HW + g * KK * HW,
                                [(RH * W, M), (HW, KK), (1, RH * W)],
                            ),
                        )
                # xh gather on the scalar ring (emitted after all ACT evacs so
                # its sem-waits don't block activation work in the ACT FIFO)
                xh_ap = xh[:, :]
                P_XH = xh_ap.ap[0][0]
                for s in range(S):
                    for g in range(G):
                        pb = (s * G + g) * M
                        nc.scalar.dma_start(
                            out=_ap(
                                xh_ap,
                                pb * P_XH,
                                [(P_XH, M), (HR * WP, GC), (1, HR * WP)],
                            ),
                            in_=_ap(
                                o1d_ap,
                                s * CMID * PLANE + (g * GC) * PLANE,
                                [(RH * WP, M), (PLANE, GC), (1, HR * WP)],
                            ),
                        )
                # xh2 = xh shifted right one element (xh2[:, 0] never read)
                xh2_ap = xh2[:, :]
                P_XH2 = xh2_ap.ap[0][0]
                nc.scalar.dma_start(
                    out=_ap(xh2_ap, 1, [(P_XH2, NP_INV), (1, XHF - 1)]),
                    in_=_ap(xh_ap, 0, [(P_XH, NP_INV), (1, XHF - 1)]),
                )

            # ---- involution: DVE products + PE identity-matmul accumulate -
            xh_v = xh[:, :].rearrange("p (c r w) -> p c r w", r=HR, w=WP)
            xh2_v = xh2[:, :].rearrange("p (c r w) -> p c r w", r=HR, w=WP)
            w2t_v = w2t[:, :].rearrange("p (k r w) -> p k r w", k=KK, r=RH)
            # even-kw taps first: they only need xh, so the DVE can start
            # before the xh2 shift-copy lands
            taps = [k for k in range(KK) if (k % KS) % 2 == 0] + [
                k for k in range(KK) if (k % KS) % 2 == 1
            ]
            with tc.tile_pool(name="psa", bufs=1, space="PSUM") as pac:
                acc = pac.tile([128, NBANK * 512], dt.float32, tag="acc")
                for i, k in enumerate(taps):
                    kh, kw = divmod(k, KS)
                    if kw % 2 == 0:
                        src_v, wc = xh_v, kw
                    else:
                        src_v, wc = xh2_v, kw + 1
                    tmp = tpool.tile([NP_INV, ACCF], dt.bfloat16, tag="tmp")
                    tmp_v = tmp[:, :].rearrange("p (c r w) -> p c r w", r=RH, w=W)
                    nc.vector.tensor_mul(
                        tmp_v,
                        src_v[:, :, kh : kh + RH, wc : wc + W],
                        w2t_v[:, k : k + 1, :, :].to_broadcast([NP_INV, GC, RH, W]),
                    )
                    for n in range(NBANK):
                        nc.tensor.matmul(
                            acc[:NP_INV, n * 512 : (n + 1) * 512],
                            id_sb[:NP_INV, :NP_INV],
                            tmp[:, n * 512 : (n + 1) * 512],
                            start=(i == 0),
                            stop=False,
                            skip_group_check=True,
                        )
                # BN2 bias (pre-divided by scale) via a K=4 matmul: the
                # group-indicator stationary routes pattern g to the
                # partitions of group g
                for n in range(NBANK):
                    nc.tensor.matmul(
                        acc[:NP_INV, n * 512 : (n + 1) * 512],
                        gsel_sb[:, :NP_INV],
                        b2pat_sb[:, n * 512 : (n + 1) * 512],
                        start=False,
                        stop=True,
                        skip_group_check=True,
                    )
                # evac IS the BN2 ReLU (scale folded into conv3 weights)
                nc.scalar.activation(accsb[:, :], acc[:NP_INV, :ACCF], AF.Relu)

            # ---- corner-turn back to channel partitions -------------------
            accd = dpool.tile([NP_INV, ACCF], dt.bfloat16, tag="accd")
            accd_ap = accd[:, :]
            nc.sync.dma_start(out=accd_ap, in_=accsb[:, :])
            out2_ap = out2[:, :]
            P_O2 = out2_ap.ap[0][0]
            for s in range(S):
                for g in range(G):
                    pb = (s * G + g) * M
                    nc.sync.dma_start(
                        out=_ap(
                            out2_ap,
                            (g * GC) * P_O2 + s * HW,
                            [(P_O2, GC), (RH * W, M), (1, RH * W)],
                        ),
                        in_=_ap(
                            accd_ap,
                            pb * ACCF,
                            [(RH * W, GC), (ACCF, M), (1, RH * W)],
                        ),
                    )

            # ---- conv3 + BN3 + residual + ReLU -> out ---------------------
            w3t_v = w3t_sb[:, :].rearrange("p (k c) -> p k c", k=2)
            with tc.tile_pool(name="ps3", bufs=4, space="PSUM") as p3:
                for s in range(S):
                    for oc in range(2):
                        for n in range(NCH):
                            ps = p3.tile([128, NW], dt.float32, tag="ps3")
                            nc.tensor.matmul(
                                ps[:, :],
                                w3t_v[:, oc, :],
                                out2[:, s * HW + n * NW : s * HW + (n + 1) * NW],
                                start=True,
                                stop=False,
                            )
                            nc.tensor.matmul(
                                ps[:, :],
                                id_sb[:, :],
                                xbf_v[:, s, oc, n * NW : (n + 1) * NW],
                                start=False,
                                stop=True,
                            )
                            ob = spool.tile([128, NW], dt.float32, tag="obuf")
                            nc.scalar.activation(
                                ob[:, :], ps[:, :], AF.Relu, bias=b3_sb[:, oc : oc + 1]
                            )
                            nc.sync.dma_start(
                                out=out.ap()[
                                    s, oc * 128 : (oc + 1) * 128, n * NW : (n + 1) * NW
                                ],
                                in_=ob[:, :],
                            )

    nc.compile()
    _CACHE["nc"] = nc
    return nc


def _f32(a):
    return np.ascontiguousarray(a, dtype=np.float32)


def prep_weights(inputs):
    """Host-side folding of BN scales into conv weights; bf16 casts."""
    f = inputs
    s1 = f["bn1_g"] / np.sqrt(f["bn1_v"] + EPS)
    b1_eff = f["bn1_b"] - f["bn1_m"] * s1
    w1t_eff = (_f32(f["conv1_w"]) * s1[:, None]).T          # [256, 64]

    si = f["inv_bn_g"] / np.sqrt(f["inv_bn_v"] + EPS)
    bi_eff = f["inv_bn_b"] - f["inv_bn_m"] * si
    c1t_eff = (_f32(f["inv_c1_w"]) * si[:, None]).T         # [64, 16]

    c2t_eff = _f32(f["inv_c2_w"]).T                         # [16, 196]
    b2c = _f32(f["inv_c2_b"])

    # relu(s2*y + b2n) = s2 * relu(y + b2n/s2), valid because s2 > 0: the
    # scale folds into conv3's input columns, the shifted bias is added in
    # PSUM by the gsel/b2pat matmul, and the accumulator evac applies the relu
    s2 = _f32(f["bn2_g"] / np.sqrt(f["bn2_v"] + EPS))
    b2n = _f32(f["bn2_b"] - f["bn2_m"] * s2)
    s3 = f["bn3_g"] / np.sqrt(f["bn3_v"] + EPS)
    b3_eff = f["bn3_b"] - f["bn3_m"] * s3
    w3_eff = _f32(f["conv3_w"]) * s3[:, None] * s2[None, :]  # [256, 64]
    w3t_eff = w3_eff.T                                       # [64, 256]

    d = {}
    d["w1t"] = np.ascontiguousarray(
        w1t_eff.reshape(2, 128, CMID).astype(BF16)
    )
    d["b1"] = _f32(b1_eff)[:, None]
    d["c1t"] = np.ascontiguousarray(c1t_eff.astype(BF16))
    d["bi"] = _f32(bi_eff)[:, None]
    d["c2ta"] = np.ascontiguousarray(c2t_eff[:, 0:98].astype(BF16))
    d["c2tb"] = np.ascontiguousarray(c2t_eff[:, 98:196].astype(BF16))
    d["b2a"] = _f32(b2c[0:98])[:, None]
    d["b2b"] = _f32(b2c[98:196])[:, None]
    d["w3t"] = np.ascontiguousarray(
        w3t_eff.reshape(CMID, 2, 128).transpose(1, 0, 2).astype(BF16)
    )
    d["b3"] = _f32(b3_eff.reshape(2, 128).T)
    d["ident"] = np.ascontiguousarray(np.eye(128, dtype=np.float32).astype(BF16))
    b2r = b2n / s2           # relu-shifted BN2 bias per mid channel [64]
    # pattern for group g over the (c, r, w) free dim: b2r[g*16+c]
    d["b2pat"] = np.ascontiguousarray(
        np.repeat(b2r.reshape(G, GC), RH * W, axis=1).astype(BF16)
    )
    gs = np.zeros((G, 128), np.float32)
    for p in range(NP_INV):
        gs[(p // M) % G, p] = 1.0
    d["gsel"] = np.ascontiguousarray(gs.astype(BF16))
    return d


def make_in_maps(inputs):
    prep = prep_weights(inputs)
    x = _f32(inputs["x"]).reshape(16, CIN, HW)
    in_maps = []
    for i in range(N_CORES):
        m = dict(prep)
        m["xin"] = np.ascontiguousarray(x[S * i : S * i + S])
        in_maps.append(m)
    return in_maps


def kernel(**inputs):
    from concourse.bass_utils import run_bass_kernel_spmd

    nc = build_module()
    in_maps = make_in_maps(inputs)
    res = run_bass_kernel_spmd(nc, in_maps, core_ids=list(range(N_CORES)))
    outs = [res.results[i]["out"].reshape(S, CIN, H, W) for i in range(N_CORES)]
    return np.concatenate(outs, axis=0).astype(np.float32)
